# revision 18
# baseline (speedup 1.0000x reference)
"""MoE LoRA linear layer kernel for Trainium2, data-parallel over 8 NeuronCores.

Math (per token n):
    down = h @ down_w.T                      [N, 64]
    mask[n, r] = val[n, k] if idx[n, k] == r else 0   (indices distinct per row)
    out = (down * mask) @ up_w.T             [N, 4096]

Sharding: tokens split 8 ways (2048/core); LoRA weights replicated.

Per-core pipeline (token tile TT=256 = 2 chunks of 128):
  1. load h in natural layout [128, 4096] per chunk (16KB DMA descriptors;
     a strided transpose-load would be 512B/descriptor and bottleneck the
     sync engine on descriptor generation)
  2. PE-transpose the 32 [128,128] blocks of each chunk into hT, copies
     alternating DVE/ACT
  3. 32 matmuls accumulate downT = dwT.T @ hT into PSUM [64, 256]
  4. build the top-k scatter mask [128, 64] per chunk on DVE
     (iota is_equal idx_k * val_k, accumulated), PE-transpose it,
     multiply with downT -> resT
  5. up-proj per chunk: 8x matmul [K=64, M=128, N=512] -> psum, assemble
     out_sb [128, 4096], single fat store per chunk

All small constants (dwT, identity, iota, idx, val) are host-packed into one
[128, CB] blob = single DMA.
"""

import sys

for p in ("/opt/trn_rl_repo", "/opt/pypackages"):
    if p not in sys.path:
        sys.path.insert(0, p)

import numpy as np

N, D_IN, D_OUT, RANK, TOPK = 16384, 4096, 4096, 64, 8
NCORES = 8
NT = N // NCORES          # tokens per core = 2048
P = 128                   # partitions
TT = 256                  # token tile (down-matmul free dim)
NKC = D_IN // P           # 32 contraction chunks for down proj
NJ = TT // P              # 2 x 128-token chunks per tile
NTILES = NT // TT         # 8 token tiles per core
NCHUNK = NT // P          # 16 x 128-token chunks per core
OT = 512                  # output col tile
NOT = D_OUT // OT         # 8 output col tiles

# const blob column layout (f32, [128, CB])
C_DWT = 0                 # [128, 32*64]   dwT chunk ki at C_DWT + ki*64
C_ID = C_DWT + NKC * RANK           # [128, 128] identity
C_IOTA = C_ID + P                   # [128, 64]  iota over rank
C_IDX = C_IOTA + RANK               # [128, 16*8] idx (chunk-major)
C_VAL = C_IDX + NCHUNK * TOPK       # [128, 16*8] val
CB = C_VAL + NCHUNK * TOPK

_CACHE = {}


def _build_program():
    import concourse.bacc as bacc
    import concourse.mybir as mybir
    from concourse import tile

    f32 = mybir.dt.float32
    # Bacc (not plain Bass): its finalize() runs move_matmul_waits_to_-
    # ldweights + generate_event_semaphores, which split semaphore waits to
    # satisfy the TRN2 one-wait-per-instruction constraint.
    nc = bacc.Bacc()

    h = nc.declare_dram_parameter("h", [NT, D_IN], f32, isOutput=False)
    cblob = nc.declare_dram_parameter("cblob", [P, CB], f32, isOutput=False)
    upw = nc.declare_dram_parameter("upw", [RANK, D_OUT], f32, isOutput=False)
    out = nc.declare_dram_parameter("out", [NT, D_OUT], f32, isOutput=True)

    eq = mybir.AluOpType.is_equal
    mult = mybir.AluOpType.mult

    with tile.TileContext(nc) as tc:
        with (
            tc.tile_pool(name="const", bufs=1) as const,
            tc.tile_pool(name="hnat", bufs=3) as hnat_pool,
            tc.tile_pool(name="hT", bufs=2) as hT_pool,
            tc.tile_pool(name="mask", bufs=4) as mask_pool,
            tc.tile_pool(name="resT", bufs=2) as resT_pool,
            tc.tile_pool(name="outsb", bufs=2) as out_pool,
            tc.tile_pool(name="psum_h", bufs=2, space="PSUM") as psum_h_pool,
            tc.tile_pool(name="psum_dn", bufs=2, space="PSUM") as psum_dn_pool,
            tc.tile_pool(name="psum_up", bufs=3, space="PSUM") as psum_up_pool,
        ):
            cb = const.tile([P, CB], f32)
            upT = const.tile([RANK, D_OUT], f32)

            nc.sync.dma_start(out=cb[:], in_=cblob[:, :])
            nc.sync.dma_start(out=upT[:], in_=upw[:, :])

            dwT = cb[:, C_DWT:C_DWT + NKC * RANK]
            ident = cb[:, C_ID:C_ID + P]
            iota_sb = cb[:, C_IOTA:C_IOTA + RANK]
            idx_sb = cb[:, C_IDX:C_IDX + NCHUNK * TOPK]
            val_sb = cb[:, C_VAL:C_VAL + NCHUNK * TOPK]

            copy_engines = [nc.vector.tensor_copy, nc.scalar.copy]

            for tt in range(NTILES):
                # 1. natural-layout loads, one per 128-token chunk
                h_nats = []
                for j in range(NJ):
                    h_nat = hnat_pool.tile([P, D_IN], f32)
                    row = tt * TT + j * P
                    nc.sync.dma_start(out=h_nat[:], in_=h[row:row + P, :])
                    h_nats.append(h_nat)

                # 2. PE-transpose h blocks into hT (free layout: ki-major,
                #    token-minor so each down matmul reads [128, TT])
                hT = hT_pool.tile([P, NKC * TT], f32)
                for ki in range(NKC):
                    for j in range(NJ):
                        psum_h = psum_h_pool.tile([P, P], f32)
                        nc.tensor.transpose(
                            psum_h[:],
                            h_nats[j][:, ki * P:(ki + 1) * P],
                            ident[:],
                        )
                        cp = copy_engines[(ki * NJ + j) % 2]
                        cp(
                            out=hT[:, ki * TT + j * P:ki * TT + (j + 1) * P],
                            in_=psum_h[:],
                        )

                # 3. down projection, accumulated over NKC chunks
                psum_dn = psum_dn_pool.tile([RANK, TT], f32)
                for ki in range(NKC):
                    nc.tensor.matmul(
                        psum_dn[:],
                        lhsT=dwT[:, ki * RANK:(ki + 1) * RANK],
                        rhs=hT[:, ki * TT:(ki + 1) * TT],
                        start=(ki == 0),
                        stop=(ki == NKC - 1),
                    )

                # psum_dn -> SBUF so the mask multiply has one PSUM operand
                # and the mask transpose's deps stay DVE-only
                down_sb = resT_pool.tile([RANK, TT], f32, tag="down_sb")
                nc.scalar.copy(out=down_sb[:], in_=psum_dn[:])

                resT = resT_pool.tile([RANK, TT], f32)
                for j in range(NJ):
                    jj = tt * NJ + j
                    # 4. top-k scatter mask for these 128 tokens
                    mask = mask_pool.tile([P, RANK], f32)
                    acc = mask_pool.tile([P, RANK], f32, tag="maskacc")
                    for k in range(TOPK):
                        col = jj * TOPK + k
                        dst = acc if k == 0 else mask
                        nc.vector.tensor_scalar(
                            out=dst[:],
                            in0=iota_sb[:],
                            scalar1=idx_sb[:, col:col + 1],
                            scalar2=val_sb[:, col:col + 1],
                            op0=eq,
                            op1=mult,
                        )
                        if k > 0:
                            nc.vector.tensor_add(acc[:], acc[:], mask[:])

                    psum_tr = psum_h_pool.tile([RANK, P], f32, tag="psum_h")
                    nc.tensor.transpose(psum_tr[:], acc[:], ident[:])
                    nc.vector.tensor_mul(
                        resT[:, j * P:(j + 1) * P],
                        down_sb[:, j * P:(j + 1) * P],
                        psum_tr[:],
                    )

                    # 5. up projection + fat store
                    out_sb = out_pool.tile([P, D_OUT], f32)
                    for o in range(NOT):
                        psum_up = psum_up_pool.tile([P, OT], f32)
                        nc.tensor.matmul(
                            psum_up[:],
                            lhsT=resT[:, j * P:(j + 1) * P],
                            rhs=upT[:, o * OT:(o + 1) * OT],
                            start=True,
                            stop=True,
                        )
                        cp = copy_engines[o % 2]
                        cp(
                            out=out_sb[:, o * OT:(o + 1) * OT],
                            in_=psum_up[:],
                        )
                    nc.sync.dma_start(
                        out=out[jj * P:(jj + 1) * P, :],
                        in_=out_sb[:],
                    )

    # Run the Bacc pipeline (register alloc + wait splitting for the TRN2
    # one-wait-per-instruction constraint) before the module is serialized.
    nc.finalize()
    return nc


def _get_program():
    if "nc" not in _CACHE:
        _CACHE["nc"] = _build_program()
    return _CACHE["nc"]


def prepare_in_maps(hidden_states, down_w, up_w, top_k_values, top_k_indices):
    h = np.ascontiguousarray(hidden_states, dtype=np.float32)
    dw = np.ascontiguousarray(down_w, dtype=np.float32)
    uw = np.ascontiguousarray(up_w, dtype=np.float32)
    vals = np.ascontiguousarray(top_k_values, dtype=np.float32)
    idxf = top_k_indices.astype(np.float32)

    upT = np.ascontiguousarray(uw.T)  # [64, 4096]

    # dwT[i, kc*64 + r] = dw[r, kc*128 + i]
    dwT = dw.reshape(RANK, NKC, P).transpose(2, 1, 0).reshape(P, NKC * RANK)
    ident = np.eye(P, dtype=np.float32)
    iota = np.broadcast_to(np.arange(RANK, dtype=np.float32), (P, RANK))

    in_maps = []
    for c in range(NCORES):
        s = slice(c * NT, (c + 1) * NT)
        # idx/val packed [p, chunk*8 + k] for this core's 16 chunks
        idx_p = idxf[s].reshape(NCHUNK, P, TOPK).transpose(1, 0, 2).reshape(P, -1)
        val_p = vals[s].reshape(NCHUNK, P, TOPK).transpose(1, 0, 2).reshape(P, -1)
        cb = np.concatenate([dwT, ident, iota, idx_p, val_p], axis=1)
        assert cb.shape == (P, CB)
        in_maps.append(
            {
                "h": h[s],
                "cblob": np.ascontiguousarray(cb),
                "upw": upT,
            }
        )
    return in_maps


def kernel(hidden_states, down_w, up_w, top_k_values, top_k_indices, **_kw):
    from concourse.bass_utils import run_bass_kernel_spmd

    nc = _get_program()
    in_maps = prepare_in_maps(
        hidden_states, down_w, up_w, top_k_values, top_k_indices
    )
    res = run_bass_kernel_spmd(nc, in_maps, core_ids=list(range(NCORES)))
    return np.concatenate([r["out"] for r in res.results], axis=0)
